# revision 24
# baseline (speedup 1.0000x reference)
"""CALoraLinear kernel for 8 TRN2 NeuronCores (Bass/Tile, SPMD).

Math (derived from the reference):
  orig = x @ W.T + bias
  top2 classes c1,c2 per row from pseudo_index[b, :64]
  g_j = <lora_A[c_j], x[b]>          (only rows 0..63 of lora_A are reachable)
  lora_out[b,o] = 16 * sum_c mask[b,c] * G[b,c] * lora_B[o,c]
  out = orig + lora_out + bias       (bias added twice)

Sharding: column-shard W across the 8 cores (each core owns 512 output
columns, full batch); x / lora_A / pseudo_index replicated. Host
concatenates the per-core [512, 512] blocks along the output axis.
(An 8-core G k-split with a DRAM AllReduce was tried and rejected: the
collective measures ~18us internally and starts tens of us late in this
runtime, and enabling collectives adds a global barrier to the preamble.)

Schedule: fp16 operand stream (PE upconverts to FP22; ~3e-4 rel err,
half the DMA bytes of f32r). Two single-k-tile chunks lead so the PE
starts ~1us earlier, then 15 double-k-tile chunks. All input DMA
triggers are front-loaded on the two HWDGE rings; pp (pseudo_index +
lora_B block) is sequenced mid-stream where it doesn't gate anything.
Dummy matmuls on an uninitialized tile warm the PE clock out of its
cold p-state during the first-chunk DMA wait. G accumulates unpacked in
one PSUM tile (G-before-mains per k-tile, so G closes one main-matmul
early); ht multiplies the G PSUM directly with the top-2 mask. The tail
matmuls close each PSUM bank with stop=True and each bank's copy-out +
store DMA pipeline against the next bank's tail matmul on alternating
rings.

fp8 was evaluated and rejected: e4m3 quantization of x and W measures
3.4e-2 full-output rel err, over the 2e-2 gate.
"""

import os
import sys

for _p in ("/opt/trn_rl_repo",):
    if _p not in sys.path:
        sys.path.insert(0, _p)

import numpy as np

import concourse.bass as bass
import concourse.bacc as bacc
import concourse.mybir as mybir
from concourse.tile import TileContext, add_dep_helper
from concourse.bass_utils import run_bass_kernel_spmd


def _ensure_ntff_hook_module():
    """run_bass_kernel_spmd(trace=True) imports antenv.axon_hooks, which the
    agent image's antenv package lacks. Provide it (and register the real
    ctypes NTFF hook when available) so a tracing caller doesn't crash."""
    import types

    try:
        import antenv
    except ImportError:
        return
    if getattr(antenv, "axon_hooks", None) is not None:
        return
    mod = types.ModuleType("antenv.axon_hooks")
    state = {"hook": None}
    mod.set_axon_ntff_profile_hook = lambda h: state.__setitem__("hook", h)
    mod.get_axon_ntff_profile_hook = lambda: state["hook"]
    sys.modules["antenv.axon_hooks"] = mod
    antenv.axon_hooks = mod
    try:
        from trn_agent_boot.trn_boot import _ntff_profile_via_ctypes

        mod.set_axon_ntff_profile_hook(
            _ntff_profile_via_ctypes("/opt/axon/libaxon_pjrt.so")
        )
    except Exception:
        pass


_ensure_ntff_hook_module()

B, IN, OUT = 512, 4096, 4096
NUM_CLASS, RANK = 64, 8
NCORES = 8
OUT_L = OUT // NCORES  # 512
P = 128
KT = IN // P           # 32 k-tiles
BT = B // P            # 4 batch tiles

NSING = 2                  # leading single-k-tile chunks
NDBL = (KT - NSING) // 2   # 15 double-k-tile chunks

# single chunk columns: [x: B][w: OUT_L][a: 64]
S_XOFF, S_WOFF, S_AOFF = 0, B, B + OUT_L
SW = B + OUT_L + NUM_CLASS                 # 1088
# double chunk columns: [x0][x1][w0][w1][a0][a1]
D_XOFF, D_WOFF, D_AOFF = 0, 2 * B, 2 * (B + OUT_L)
DW = 2 * SW                                # 2176

# pp layout: [ps: BT*64][psT: B][bS: OUT_L (rows 0:65)]
PSOFF = 0
PTOFF = BT * NUM_CLASS
BSOFF = PTOFF + B
PPW = BSOFF + OUT_L

F32 = mybir.dt.float32
F32R = mybir.dt.float32r
F16 = mybir.dt.float16
X = mybir.AxisListType.X

NWARM = int(os.environ.get("NWARM", "6"))
PP_SLOT = int(os.environ.get("PP_SLOT", "6"))  # pp issued after this double

_cache = {}
# test.py reads this after a traced run for HW exec time
last_results = None


def _build():
    key = f"nc_w{NWARM}_p{PP_SLOT}"
    if key in _cache:
        return _cache[key]
    nc = bacc.Bacc(
        bass.get_trn_type() or "TRN2",
        target_bir_lowering=False,
        debug=False,
        num_devices=NCORES,
    )

    xw_s = nc.dram_tensor("xw_s", [NSING, P, SW], F16, kind="ExternalInput")
    xw_d = nc.dram_tensor("xw_d", [NDBL, P, DW], F16, kind="ExternalInput")
    pp = nc.dram_tensor("pp", [P, PPW], F32R, kind="ExternalInput")
    # f16 output staging: halves the PSUM->SBUF copy and store-DMA bytes on
    # the critical tail; the host upcasts. Adds ~2^-12 RMS rounding on top
    # of the fp16 stream's ~3e-4 rel err (measured total 3.9e-4).
    out = nc.dram_tensor("out", [B, OUT_L], F16, kind="ExternalOutput")

    with TileContext(nc) as tc:
        with (
            tc.tile_pool(name="xwp", bufs=1) as xwpool,
            tc.tile_pool(name="sml", bufs=1) as spool,
            tc.tile_pool(name="tl", bufs=1) as tpool,
            tc.tile_pool(name="op", bufs=1) as opool,
            tc.tile_pool(name="dr", bufs=1, space="DRAM") as dpool,
            tc.tile_pool(name="ps", bufs=1, space="PSUM") as ppool,
        ):
            # ---- PE warmup: dummy matmuls ramp the PE clock out of its
            # cold p-state while the first chunk DMA is in flight. The
            # result bank is never read.
            if NWARM:
                wt = spool.tile([P, P + OUT_L], F16, tag="warm")
                nc.vector.memset(wt, 0.0)
                warm_ps = ppool.tile([P, OUT_L], F32, tag="warm", name="warm")
                for _ in range(NWARM):
                    nc.tensor.matmul(
                        warm_ps,
                        lhsT=wt[:, :P],
                        rhs=wt[:, P : P + OUT_L],
                        start=True,
                        stop=True,
                    )

            # ---- input DMA triggers on the two HWDGE rings, window-2 gated:
            # each ring holds at most 2 in-flight transfers. With more, the
            # DMA queues round-robin across every outstanding transfer and
            # the FIRST chunk's completion slips by many us (measured: first
            # matmul at 18.6us instead of ~10us when all 17 were issued
            # up-front). pp is sequenced mid-stream on the scalar ring: its
            # consumers (top-2 mask, bS) aren't needed until stream end.
            s_tiles = [
                xwpool.tile([P, SW], F16, tag=f"s{c}", name=f"s{c}")
                for c in range(NSING)
            ]
            d_tiles = [
                xwpool.tile([P, DW], F16, tag=f"d{c}", name=f"d{c}")
                for c in range(NDBL)
            ]
            pp_sb = spool.tile([P, PPW], F32R)
            prev_dma = {0: None, 1: None}  # per-ring (c-2) gating chain
            gate_dma = {0: None, 1: None}

            def issue(eng_i, out_tile, src):
                eng = nc.sync if eng_i == 0 else nc.scalar
                dma = eng.dma_start(out=out_tile, in_=src)
                if gate_dma[eng_i] is not None:
                    add_dep_helper(
                        dma.ins,
                        gate_dma[eng_i].ins,
                        reason="window-2 DMA gating per ring",
                    )
                gate_dma[eng_i] = prev_dma[eng_i]
                prev_dma[eng_i] = dma
                return dma

            issue(0, s_tiles[0], xw_s[0])
            issue(1, s_tiles[1], xw_s[1])
            d_dmas = []
            for c in range(NDBL):
                d_dmas.append(issue(c % 2, d_tiles[c], xw_d[c]))
            # pp rides the GPSIMD SWDGE ring so it never crowds the chunk
            # stream; gated until d1 lands (its consumers run mid-stream)
            pp_dma = nc.gpsimd.dma_start(out=pp_sb, in_=pp[:, :])
            add_dep_helper(
                pp_dma.ins, d_dmas[5].ins, reason="keep pp off the early chunks"
            )

            ps_sb = pp_sb[:, PSOFF : PSOFF + BT * NUM_CLASS].bitcast(F32)
            psT_sb = pp_sb[:NUM_CLASS, PTOFF : PTOFF + B].bitcast(F32)
            bS_sb = pp_sb[:, BSOFF : BSOFF + OUT_L]  # rows 65:128 are zeros

            # ---- PSUM accumulators ----
            mps = [
                ppool.tile([P, OUT_L], F32, tag=f"main{bt}", name=f"main{bt}")
                for bt in range(BT)
            ]
            # G accumulator: full 128-partition bank; the real G lives in
            # rows 64:128. The G matmul's stationary is a 128-col window
            # ending at the a-block (64 w-cols of garbage + the 64 a-cols),
            # so its geometry matches the main matmuls exactly — a 64-wide
            # stationary forces a PE pipeline drain (~110ns) on the G matmul
            # AND on the following main (measured 333/322 vs 216ns).
            gt_ps = ppool.tile([P, B], F32, tag="gt", name="gt_ps")

            def do_k(xk, wk, a128, kidx):
                # G first: at the last k-tile this lets the DVE ht chain
                # overlap the final main matmuls
                nc.tensor.matmul(
                    gt_ps,
                    lhsT=a128,
                    rhs=xk,
                    start=(kidx == 0),
                    stop=(kidx == KT - 1),
                )
                for bt in range(BT):
                    nc.tensor.matmul(
                        mps[bt],
                        lhsT=xk[:, bt * P : (bt + 1) * P],
                        rhs=wk,
                        start=(kidx == 0),
                        stop=False,
                    )

            kidx = 0
            for s in range(NSING):
                t = s_tiles[s]
                do_k(
                    t[:, S_XOFF : S_XOFF + B],
                    t[:, S_WOFF : S_WOFF + OUT_L],
                    t[:, S_AOFF - NUM_CLASS : S_AOFF + NUM_CLASS],
                    kidx,
                )
                kidx += 1

            # ---- top-2 threshold + mask (DVE/GPSIMD, overlaps the stream) ----
            m2col = spool.tile([P, BT], F32)
            for bt in range(BT):
                pt = ps_sb[:, bt * NUM_CLASS : (bt + 1) * NUM_CLASS]
                m1 = spool.tile([P, 1], F32, tag=f"m1_{bt}")
                nc.vector.reduce_max(out=m1, in_=pt, axis=X)
                negmask = spool.tile([P, NUM_CLASS], F32, tag=f"nm_{bt}")
                # (pt >= m1) * -1e30  -> additive mask that kills the max
                nc.vector.tensor_scalar(
                    out=negmask,
                    in0=pt,
                    scalar1=m1,
                    scalar2=-1.0e30,
                    op0=mybir.AluOpType.is_ge,
                    op1=mybir.AluOpType.mult,
                )
                p2 = spool.tile([P, NUM_CLASS], F32, tag=f"p2_{bt}")
                nc.vector.tensor_tensor(
                    out=p2, in0=pt, in1=negmask, op=mybir.AluOpType.add
                )
                nc.vector.reduce_max(out=m2col[:, bt : bt + 1], in_=p2, axis=X)

            # threshold shuffle on the GPSIMD (SWDGE) path: partition->free
            # [128, BT] -> flat [B] via a DRAM bounce, then broadcast-read
            # across 64 partitions.
            m2d = dpool.tile([BT, P], F32)
            nc.gpsimd.dma_start(out=m2d.rearrange("bt p -> p bt"), in_=m2col[:, :])
            thr_sb = spool.tile([NUM_CLASS, B], F32)
            nc.gpsimd.dma_start(
                out=thr_sb,
                in_=m2d.rearrange("bt p -> (bt p)")[None, :].broadcast_to(
                    [NUM_CLASS, B]
                ),
            )
            thr2 = tpool.tile([NUM_CLASS, B], F32)
            nc.vector.tensor_copy(out=thr2, in_=thr_sb)
            psT2 = tpool.tile([NUM_CLASS, B], F32)
            nc.vector.tensor_copy(out=psT2, in_=psT_sb)
            maskT = tpool.tile([NUM_CLASS, B], F32)
            nc.vector.tensor_tensor(
                out=maskT, in0=psT2, in1=thr2, op=mybir.AluOpType.is_ge
            )
            # ht padded to a full 128-row contraction (rows 65:128 zero, with
            # matching zero rows in bS) so the tail matmul keeps the same
            # 128-partition geometry as the stream and avoids the PE drain.
            ht = tpool.tile([P, B], F32R)
            # zero rows 64:128 (aligned base), then the ones row at 64
            # (Memset can't write float32r, hence the in0*0+c idiom)
            nc.vector.tensor_scalar(
                out=ht[NUM_CLASS:P, :],
                in0=psT2[0 : P - NUM_CLASS, :],
                scalar1=0.0,
                scalar2=0.0,
                op0=mybir.AluOpType.mult,
                op1=mybir.AluOpType.add,
            )
            nc.vector.tensor_scalar(
                out=ht[NUM_CLASS : NUM_CLASS + 1, :],
                in0=thr2[0:1, :],
                scalar1=0.0,
                scalar2=1.0,
                op0=mybir.AluOpType.mult,
                op1=mybir.AluOpType.add,
            )

            # ---- double chunks ----
            for c in range(NDBL):
                t = d_tiles[c]
                for kk in range(2):
                    do_k(
                        t[:, D_XOFF + kk * B : D_XOFF + (kk + 1) * B],
                        t[:, D_WOFF + kk * OUT_L : D_WOFF + (kk + 1) * OUT_L],
                        t[
                            :,
                            D_AOFF
                            + (kk - 1) * NUM_CLASS : D_AOFF
                            + (kk + 1) * NUM_CLASS,
                        ],
                        kidx,
                    )
                    kidx += 1

            # ht = G * mask, reading the G accumulator PSUM directly (the
            # real G sits in rows 64:128 of the padded accumulator)
            nc.vector.tensor_tensor(
                out=ht[0:NUM_CLASS, :],
                in0=gt_ps[NUM_CLASS : 2 * NUM_CLASS, :],
                in1=maskT,
                op=mybir.AluOpType.mult,
            )

            # ---- tail + epilogue, pipelined per batch tile: the rank-65
            # LoRA matmul closes bank bt, whose copy-out + store DMA then
            # overlap the next bank's tail matmul.
            o_all = opool.tile([P, BT * OUT_L], F16)
            for bt in range(BT):
                nc.tensor.matmul(
                    mps[bt],
                    lhsT=ht[:, bt * P : (bt + 1) * P],
                    rhs=bS_sb,
                    start=False,
                    stop=True,
                )
                # copies alternate vector/scalar and the store-DMA triggers
                # spread across three engines, so the per-bank ~0.5us copy
                # and ~0.6us issue costs overlap instead of serializing
                dst = o_all[:, bt * OUT_L : (bt + 1) * OUT_L]
                if bt % 2 == 0:
                    nc.vector.tensor_copy(out=dst, in_=mps[bt])
                else:
                    nc.scalar.copy(out=dst, in_=mps[bt])
                eng = (nc.sync, nc.gpsimd, nc.sync, nc.scalar)[bt]
                eng.dma_start(
                    out=out[bt * P : (bt + 1) * P, :],
                    in_=o_all[:, bt * OUT_L : (bt + 1) * OUT_L],
                )

    nc.finalize()
    _cache[key] = nc
    return nc


def _pack_inputs(x, pseudo_index, weight, bias, lora_A, lora_B):
    """Build the per-core chunked xw buffers + replicated small inputs."""
    xT = np.ascontiguousarray(x.T).astype(np.float16)        # [IN, B]
    aT = np.ascontiguousarray(
        lora_A[:NUM_CLASS].T
    ).astype(np.float16)                                     # [IN, 64]

    pp_base = np.zeros((P, PPW), dtype=np.float32)
    pp_base[:, PSOFF : PSOFF + BT * NUM_CLASS] = (
        pseudo_index.reshape(BT, P, NUM_CLASS)
        .transpose(1, 0, 2)
        .reshape(P, BT * NUM_CLASS)
    )
    pp_base[:NUM_CLASS, PTOFF : PTOFF + B] = pseudo_index.T

    in_maps = []
    for i in range(NCORES):
        o0 = i * OUT_L
        wTi = weight[o0 : o0 + OUT_L].T.astype(np.float16)   # [IN, OUT_L]

        xws = np.empty((NSING, P, SW), dtype=np.float16)
        for s in range(NSING):
            xws[s, :, S_XOFF : S_XOFF + B] = xT[s * P : (s + 1) * P]
            xws[s, :, S_WOFF : S_WOFF + OUT_L] = wTi[s * P : (s + 1) * P]
            xws[s, :, S_AOFF : S_AOFF + NUM_CLASS] = aT[s * P : (s + 1) * P]

        xwd = np.empty((NDBL, P, DW), dtype=np.float16)
        for c in range(NDBL):
            k0 = NSING + 2 * c
            k1 = k0 + 1
            xwd[c, :, D_XOFF : D_XOFF + B] = xT[k0 * P : (k0 + 1) * P]
            xwd[c, :, D_XOFF + B : D_XOFF + 2 * B] = xT[k1 * P : (k1 + 1) * P]
            xwd[c, :, D_WOFF : D_WOFF + OUT_L] = wTi[k0 * P : (k0 + 1) * P]
            xwd[c, :, D_WOFF + OUT_L : D_WOFF + 2 * OUT_L] = wTi[
                k1 * P : (k1 + 1) * P
            ]
            xwd[c, :, D_AOFF : D_AOFF + NUM_CLASS] = aT[k0 * P : (k0 + 1) * P]
            xwd[c, :, D_AOFF + NUM_CLASS : DW] = aT[k1 * P : (k1 + 1) * P]

        ppi = pp_base.copy()
        ppi[:NUM_CLASS, BSOFF : BSOFF + OUT_L] = (
            16.0 * lora_B[o0 : o0 + OUT_L, :NUM_CLASS].T
        )
        ppi[NUM_CLASS, BSOFF : BSOFF + OUT_L] = 2.0 * bias[o0 : o0 + OUT_L]
        in_maps.append({"xw_s": xws, "xw_d": xwd, "pp": ppi})
    return in_maps


def kernel(x, pseudo_index, weight, bias, lora_A, lora_B):
    global last_results
    x = np.ascontiguousarray(np.asarray(x, dtype=np.float32))
    pseudo_index = np.ascontiguousarray(np.asarray(pseudo_index, dtype=np.float32))
    weight = np.asarray(weight, dtype=np.float32)
    bias = np.asarray(bias, dtype=np.float32)
    lora_A = np.asarray(lora_A, dtype=np.float32)
    lora_B = np.asarray(lora_B, dtype=np.float32)

    nc = _build()
    in_maps = _pack_inputs(x, pseudo_index, weight, bias, lora_A, lora_B)
    res = run_bass_kernel_spmd(nc, in_maps, list(range(NCORES)))
    last_results = res
    return np.hstack(
        [res.results[i]["out"].astype(np.float32) for i in range(NCORES)]
    )


# revision 26
# speedup vs baseline: 1.0071x; 1.0071x over previous
"""CALoraLinear kernel for 8 TRN2 NeuronCores (Bass/Tile, SPMD).

Math (derived from the reference):
  orig = x @ W.T + bias
  top2 classes c1,c2 per row from pseudo_index[b, :64]
  g_j = <lora_A[c_j], x[b]>          (only rows 0..63 of lora_A are reachable)
  lora_out[b,o] = 16 * sum_c mask[b,c] * G[b,c] * lora_B[o,c]
  out = orig + lora_out + bias       (bias added twice)

Sharding: column-shard W across the 8 cores (each core owns 512 output
columns, full batch); x / lora_A / pseudo_index replicated. Host
concatenates the per-core [512, 512] blocks along the output axis.
(An 8-core G k-split with a DRAM AllReduce was tried and rejected: the
collective measures ~18us internally and starts tens of us late in this
runtime, and enabling collectives adds a global barrier to the preamble.)

Schedule: fp16 operand stream (PE upconverts to FP22; ~3e-4 rel err,
half the DMA bytes of f32r). Two single-k-tile chunks lead so the PE
starts ~1us earlier, then 15 double-k-tile chunks. All input DMA
triggers are front-loaded on the two HWDGE rings; pp (pseudo_index +
lora_B block) is sequenced mid-stream where it doesn't gate anything.
Dummy matmuls on an uninitialized tile warm the PE clock out of its
cold p-state during the first-chunk DMA wait. G accumulates unpacked in
one PSUM tile (G-before-mains per k-tile, so G closes one main-matmul
early); ht multiplies the G PSUM directly with the top-2 mask. The tail
matmuls close each PSUM bank with stop=True and each bank's copy-out +
store DMA pipeline against the next bank's tail matmul on alternating
rings.

fp8 was evaluated and rejected: e4m3 quantization of x and W measures
3.4e-2 full-output rel err, over the 2e-2 gate.
"""

import os
import sys

for _p in ("/opt/trn_rl_repo",):
    if _p not in sys.path:
        sys.path.insert(0, _p)

import numpy as np

import concourse.bass as bass
import concourse.bacc as bacc
import concourse.mybir as mybir
from concourse.tile import TileContext, add_dep_helper
from concourse.bass_utils import run_bass_kernel_spmd


def _ensure_ntff_hook_module():
    """run_bass_kernel_spmd(trace=True) imports antenv.axon_hooks, which the
    agent image's antenv package lacks. Provide it (and register the real
    ctypes NTFF hook when available) so a tracing caller doesn't crash."""
    import types

    try:
        import antenv
    except ImportError:
        return
    if getattr(antenv, "axon_hooks", None) is not None:
        return
    mod = types.ModuleType("antenv.axon_hooks")
    state = {"hook": None}
    mod.set_axon_ntff_profile_hook = lambda h: state.__setitem__("hook", h)
    mod.get_axon_ntff_profile_hook = lambda: state["hook"]
    sys.modules["antenv.axon_hooks"] = mod
    antenv.axon_hooks = mod
    try:
        from trn_agent_boot.trn_boot import _ntff_profile_via_ctypes

        mod.set_axon_ntff_profile_hook(
            _ntff_profile_via_ctypes("/opt/axon/libaxon_pjrt.so")
        )
    except Exception:
        pass


_ensure_ntff_hook_module()

B, IN, OUT = 512, 4096, 4096
NUM_CLASS, RANK = 64, 8
NCORES = 8
OUT_L = OUT // NCORES  # 512
P = 128
KT = IN // P           # 32 k-tiles
BT = B // P            # 4 batch tiles

NSING = 2                  # leading single-k-tile chunks
NDBL = (KT - NSING) // 2   # 15 double-k-tile chunks

# single chunk columns: [x: B][w: OUT_L][a: 64]
S_XOFF, S_WOFF, S_AOFF = 0, B, B + OUT_L
SW = B + OUT_L + NUM_CLASS                 # 1088
# double chunk columns: [x0][x1][w0][w1][a0][a1]
D_XOFF, D_WOFF, D_AOFF = 0, 2 * B, 2 * (B + OUT_L)
DW = 2 * SW                                # 2176

# pp layout: [ps: BT*64][psT: B][bS: OUT_L (rows 0:65)]
PSOFF = 0
PTOFF = BT * NUM_CLASS
BSOFF = PTOFF + B
PPW = BSOFF + OUT_L

F32 = mybir.dt.float32
F32R = mybir.dt.float32r
F16 = mybir.dt.float16
X = mybir.AxisListType.X

NWARM = int(os.environ.get("NWARM", "6"))
PP_SLOT = int(os.environ.get("PP_SLOT", "6"))  # pp issued after this double

_cache = {}
# test.py reads this after a traced run for HW exec time
last_results = None


def _build():
    key = f"nc_w{NWARM}_p{PP_SLOT}"
    if key in _cache:
        return _cache[key]
    nc = bacc.Bacc(
        bass.get_trn_type() or "TRN2",
        target_bir_lowering=False,
        debug=False,
        num_devices=NCORES,
    )

    xw_s = nc.dram_tensor("xw_s", [NSING, P, SW], F16, kind="ExternalInput")
    xw_d = nc.dram_tensor("xw_d", [NDBL, P, DW], F16, kind="ExternalInput")
    pp = nc.dram_tensor("pp", [P, PPW], F32R, kind="ExternalInput")
    # f16 output staging: halves the PSUM->SBUF copy and store-DMA bytes on
    # the critical tail; the host upcasts. Adds ~2^-12 RMS rounding on top
    # of the fp16 stream's ~3e-4 rel err (measured total 3.9e-4).
    out = nc.dram_tensor("out", [B, OUT_L], F16, kind="ExternalOutput")

    with TileContext(nc) as tc:
        with (
            tc.tile_pool(name="xwp", bufs=1) as xwpool,
            tc.tile_pool(name="sml", bufs=1) as spool,
            tc.tile_pool(name="tl", bufs=1) as tpool,
            tc.tile_pool(name="op", bufs=1) as opool,
            tc.tile_pool(name="dr", bufs=1, space="DRAM") as dpool,
            tc.tile_pool(name="ps", bufs=1, space="PSUM") as ppool,
        ):
            # ---- PE warmup: dummy matmuls ramp the PE clock out of its
            # cold p-state while the first chunk DMA is in flight. The
            # result bank is never read.
            if NWARM:
                wt = spool.tile([P, P + OUT_L], F16, tag="warm")
                nc.vector.memset(wt, 0.0)
                warm_ps = ppool.tile([P, OUT_L], F32, tag="warm", name="warm")
                for _ in range(NWARM):
                    nc.tensor.matmul(
                        warm_ps,
                        lhsT=wt[:, :P],
                        rhs=wt[:, P : P + OUT_L],
                        start=True,
                        stop=True,
                    )

            # ---- input DMA triggers on the two HWDGE rings, window-2 gated:
            # each ring holds at most 2 in-flight transfers. With more, the
            # DMA queues round-robin across every outstanding transfer and
            # the FIRST chunk's completion slips by many us (measured: first
            # matmul at 18.6us instead of ~10us when all 17 were issued
            # up-front). pp is sequenced mid-stream on the scalar ring: its
            # consumers (top-2 mask, bS) aren't needed until stream end.
            s_tiles = [
                xwpool.tile([P, SW], F16, tag=f"s{c}", name=f"s{c}")
                for c in range(NSING)
            ]
            d_tiles = [
                xwpool.tile([P, DW], F16, tag=f"d{c}", name=f"d{c}")
                for c in range(NDBL)
            ]
            pp_sb = spool.tile([P, PPW], F32R)
            prev_dma = {0: None, 1: None}  # per-ring (c-2) gating chain
            gate_dma = {0: None, 1: None}

            def issue(eng_i, out_tile, src):
                eng = nc.sync if eng_i == 0 else nc.scalar
                dma = eng.dma_start(out=out_tile, in_=src)
                if gate_dma[eng_i] is not None:
                    add_dep_helper(
                        dma.ins,
                        gate_dma[eng_i].ins,
                        reason="window-2 DMA gating per ring",
                    )
                gate_dma[eng_i] = prev_dma[eng_i]
                prev_dma[eng_i] = dma
                return dma

            issue(0, s_tiles[0], xw_s[0])
            issue(1, s_tiles[1], xw_s[1])
            d_dmas = []
            for c in range(NDBL):
                d_dmas.append(issue(c % 2, d_tiles[c], xw_d[c]))
            # pp rides the GPSIMD SWDGE ring so it never crowds the chunk
            # stream; gated until d1 lands (its consumers run mid-stream)
            pp_dma = nc.gpsimd.dma_start(out=pp_sb, in_=pp[:, :])
            add_dep_helper(
                pp_dma.ins, d_dmas[1].ins, reason="keep pp off the early chunks"
            )

            ps_sb = pp_sb[:, PSOFF : PSOFF + BT * NUM_CLASS].bitcast(F32)
            psT_sb = pp_sb[:NUM_CLASS, PTOFF : PTOFF + B].bitcast(F32)
            bS_sb = pp_sb[:, BSOFF : BSOFF + OUT_L]  # rows 65:128 are zeros

            # ---- PSUM accumulators ----
            mps = [
                ppool.tile([P, OUT_L], F32, tag=f"main{bt}", name=f"main{bt}")
                for bt in range(BT)
            ]
            # G accumulator: full 128-partition bank; the real G lives in
            # rows 64:128. The G matmul's stationary is a 128-col window
            # ending at the a-block (64 w-cols of garbage + the 64 a-cols),
            # so its geometry matches the main matmuls exactly — a 64-wide
            # stationary forces a PE pipeline drain (~110ns) on the G matmul
            # AND on the following main (measured 333/322 vs 216ns).
            gt_ps = ppool.tile([P, B], F32, tag="gt", name="gt_ps")

            def do_k(xk, wk, a128, kidx):
                # G first: at the last k-tile this lets the DVE ht chain
                # overlap the final main matmuls
                nc.tensor.matmul(
                    gt_ps,
                    lhsT=a128,
                    rhs=xk,
                    start=(kidx == 0),
                    stop=(kidx == KT - 1),
                )
                for bt in range(BT):
                    nc.tensor.matmul(
                        mps[bt],
                        lhsT=xk[:, bt * P : (bt + 1) * P],
                        rhs=wk,
                        start=(kidx == 0),
                        stop=False,
                    )

            kidx = 0
            for s in range(NSING):
                t = s_tiles[s]
                do_k(
                    t[:, S_XOFF : S_XOFF + B],
                    t[:, S_WOFF : S_WOFF + OUT_L],
                    t[:, S_AOFF - NUM_CLASS : S_AOFF + NUM_CLASS],
                    kidx,
                )
                kidx += 1

            # ---- top-2 threshold + mask (DVE/GPSIMD, overlaps the stream) ----
            m2col = spool.tile([P, BT], F32)
            for bt in range(BT):
                pt = ps_sb[:, bt * NUM_CLASS : (bt + 1) * NUM_CLASS]
                m1 = spool.tile([P, 1], F32, tag=f"m1_{bt}")
                nc.vector.reduce_max(out=m1, in_=pt, axis=X)
                negmask = spool.tile([P, NUM_CLASS], F32, tag=f"nm_{bt}")
                # (pt >= m1) * -1e30  -> additive mask that kills the max
                nc.vector.tensor_scalar(
                    out=negmask,
                    in0=pt,
                    scalar1=m1,
                    scalar2=-1.0e30,
                    op0=mybir.AluOpType.is_ge,
                    op1=mybir.AluOpType.mult,
                )
                p2 = spool.tile([P, NUM_CLASS], F32, tag=f"p2_{bt}")
                nc.vector.tensor_tensor(
                    out=p2, in0=pt, in1=negmask, op=mybir.AluOpType.add
                )
                nc.vector.reduce_max(out=m2col[:, bt : bt + 1], in_=p2, axis=X)

            # threshold shuffle on the GPSIMD (SWDGE) path: partition->free
            # [128, BT] -> flat [B] via a DRAM bounce, then broadcast-read
            # across 64 partitions.
            m2d = dpool.tile([BT, P], F32)
            nc.gpsimd.dma_start(out=m2d.rearrange("bt p -> p bt"), in_=m2col[:, :])
            thr_sb = spool.tile([NUM_CLASS, B], F32)
            nc.gpsimd.dma_start(
                out=thr_sb,
                in_=m2d.rearrange("bt p -> (bt p)")[None, :].broadcast_to(
                    [NUM_CLASS, B]
                ),
            )
            thr2 = tpool.tile([NUM_CLASS, B], F32)
            nc.vector.tensor_copy(out=thr2, in_=thr_sb)
            psT2 = tpool.tile([NUM_CLASS, B], F32)
            nc.vector.tensor_copy(out=psT2, in_=psT_sb)
            maskT = tpool.tile([NUM_CLASS, B], F32)
            nc.vector.tensor_tensor(
                out=maskT, in0=psT2, in1=thr2, op=mybir.AluOpType.is_ge
            )
            # ht padded to a full 128-row contraction (rows 65:128 zero, with
            # matching zero rows in bS) so the tail matmul keeps the same
            # 128-partition geometry as the stream and avoids the PE drain.
            ht = tpool.tile([P, B], F32R)
            # zero rows 64:128 (aligned base), then the ones row at 64
            # (Memset can't write float32r, hence the in0*0+c idiom)
            nc.vector.tensor_scalar(
                out=ht[NUM_CLASS:P, :],
                in0=psT2[0 : P - NUM_CLASS, :],
                scalar1=0.0,
                scalar2=0.0,
                op0=mybir.AluOpType.mult,
                op1=mybir.AluOpType.add,
            )
            nc.vector.tensor_scalar(
                out=ht[NUM_CLASS : NUM_CLASS + 1, :],
                in0=thr2[0:1, :],
                scalar1=0.0,
                scalar2=1.0,
                op0=mybir.AluOpType.mult,
                op1=mybir.AluOpType.add,
            )

            # ---- double chunks ----
            for c in range(NDBL):
                t = d_tiles[c]
                for kk in range(2):
                    do_k(
                        t[:, D_XOFF + kk * B : D_XOFF + (kk + 1) * B],
                        t[:, D_WOFF + kk * OUT_L : D_WOFF + (kk + 1) * OUT_L],
                        t[
                            :,
                            D_AOFF
                            + (kk - 1) * NUM_CLASS : D_AOFF
                            + (kk + 1) * NUM_CLASS,
                        ],
                        kidx,
                    )
                    kidx += 1

            # ht = G * mask, reading the G accumulator PSUM directly (the
            # real G sits in rows 64:128 of the padded accumulator)
            nc.vector.tensor_tensor(
                out=ht[0:NUM_CLASS, :],
                in0=gt_ps[NUM_CLASS : 2 * NUM_CLASS, :],
                in1=maskT,
                op=mybir.AluOpType.mult,
            )

            # ---- tail + epilogue, pipelined per batch tile: the rank-65
            # LoRA matmul closes bank bt, whose copy-out + store DMA then
            # overlap the next bank's tail matmul.
            o_all = opool.tile([P, BT * OUT_L], F16)
            for bt in range(BT):
                nc.tensor.matmul(
                    mps[bt],
                    lhsT=ht[:, bt * P : (bt + 1) * P],
                    rhs=bS_sb,
                    start=False,
                    stop=True,
                )
                nc.vector.tensor_copy(
                    out=o_all[:, bt * OUT_L : (bt + 1) * OUT_L], in_=mps[bt]
                )
                # spread store-DMA triggers across three engines so their
                # ~0.6us issue costs run in parallel
                eng = (nc.sync, nc.scalar, nc.gpsimd, nc.sync)[bt]
                eng.dma_start(
                    out=out[bt * P : (bt + 1) * P, :],
                    in_=o_all[:, bt * OUT_L : (bt + 1) * OUT_L],
                )

    nc.finalize()
    _cache[key] = nc
    return nc


def _pack_inputs(x, pseudo_index, weight, bias, lora_A, lora_B):
    """Build the per-core chunked xw buffers + replicated small inputs."""
    xT = np.ascontiguousarray(x.T).astype(np.float16)        # [IN, B]
    aT = np.ascontiguousarray(
        lora_A[:NUM_CLASS].T
    ).astype(np.float16)                                     # [IN, 64]

    pp_base = np.zeros((P, PPW), dtype=np.float32)
    pp_base[:, PSOFF : PSOFF + BT * NUM_CLASS] = (
        pseudo_index.reshape(BT, P, NUM_CLASS)
        .transpose(1, 0, 2)
        .reshape(P, BT * NUM_CLASS)
    )
    pp_base[:NUM_CLASS, PTOFF : PTOFF + B] = pseudo_index.T

    in_maps = []
    for i in range(NCORES):
        o0 = i * OUT_L
        wTi = weight[o0 : o0 + OUT_L].T.astype(np.float16)   # [IN, OUT_L]

        xws = np.empty((NSING, P, SW), dtype=np.float16)
        for s in range(NSING):
            xws[s, :, S_XOFF : S_XOFF + B] = xT[s * P : (s + 1) * P]
            xws[s, :, S_WOFF : S_WOFF + OUT_L] = wTi[s * P : (s + 1) * P]
            xws[s, :, S_AOFF : S_AOFF + NUM_CLASS] = aT[s * P : (s + 1) * P]

        xwd = np.empty((NDBL, P, DW), dtype=np.float16)
        for c in range(NDBL):
            k0 = NSING + 2 * c
            k1 = k0 + 1
            xwd[c, :, D_XOFF : D_XOFF + B] = xT[k0 * P : (k0 + 1) * P]
            xwd[c, :, D_XOFF + B : D_XOFF + 2 * B] = xT[k1 * P : (k1 + 1) * P]
            xwd[c, :, D_WOFF : D_WOFF + OUT_L] = wTi[k0 * P : (k0 + 1) * P]
            xwd[c, :, D_WOFF + OUT_L : D_WOFF + 2 * OUT_L] = wTi[
                k1 * P : (k1 + 1) * P
            ]
            xwd[c, :, D_AOFF : D_AOFF + NUM_CLASS] = aT[k0 * P : (k0 + 1) * P]
            xwd[c, :, D_AOFF + NUM_CLASS : DW] = aT[k1 * P : (k1 + 1) * P]

        ppi = pp_base.copy()
        ppi[:NUM_CLASS, BSOFF : BSOFF + OUT_L] = (
            16.0 * lora_B[o0 : o0 + OUT_L, :NUM_CLASS].T
        )
        ppi[NUM_CLASS, BSOFF : BSOFF + OUT_L] = 2.0 * bias[o0 : o0 + OUT_L]
        in_maps.append({"xw_s": xws, "xw_d": xwd, "pp": ppi})
    return in_maps


def kernel(x, pseudo_index, weight, bias, lora_A, lora_B):
    global last_results
    x = np.ascontiguousarray(np.asarray(x, dtype=np.float32))
    pseudo_index = np.ascontiguousarray(np.asarray(pseudo_index, dtype=np.float32))
    weight = np.asarray(weight, dtype=np.float32)
    bias = np.asarray(bias, dtype=np.float32)
    lora_A = np.asarray(lora_A, dtype=np.float32)
    lora_B = np.asarray(lora_B, dtype=np.float32)

    nc = _build()
    in_maps = _pack_inputs(x, pseudo_index, weight, bias, lora_A, lora_B)
    res = run_bass_kernel_spmd(nc, in_maps, list(range(NCORES)))
    last_results = res
    return np.hstack(
        [res.results[i]["out"].astype(np.float32) for i in range(NCORES)]
    )


# revision 28
# speedup vs baseline: 1.0772x; 1.0697x over previous
"""CALoraLinear kernel for 8 TRN2 NeuronCores (Bass/Tile, SPMD).

Math (derived from the reference):
  orig = x @ W.T + bias
  top2 classes c1,c2 per row from pseudo_index[b, :64]
  g_j = <lora_A[c_j], x[b]>          (only rows 0..63 of lora_A are reachable)
  lora_out[b,o] = 16 * sum_c mask[b,c] * G[b,c] * lora_B[o,c]
  out = orig + lora_out + bias       (bias added twice)

Sharding: column-shard W across the 8 cores (each core owns 512 output
columns, full batch); x / lora_A / pseudo_index replicated. Host
concatenates the per-core [512, 512] blocks along the output axis.
(An 8-core G k-split with a DRAM AllReduce was tried and rejected: the
collective measures ~18us internally and starts tens of us late in this
runtime, and enabling collectives adds a global barrier to the preamble.)

Schedule: fp16 operand stream (PE upconverts to FP22; ~3e-4 rel err,
half the DMA bytes of f32r). Two single-k-tile chunks lead so the PE
starts ~1us earlier, then 15 double-k-tile chunks. All input DMA
triggers are front-loaded on the two HWDGE rings; pp (pseudo_index +
lora_B block) is sequenced mid-stream where it doesn't gate anything.
Dummy matmuls on an uninitialized tile warm the PE clock out of its
cold p-state during the first-chunk DMA wait. G accumulates unpacked in
one PSUM tile (G-before-mains per k-tile, so G closes one main-matmul
early); ht multiplies the G PSUM directly with the top-2 mask. The tail
matmuls close each PSUM bank with stop=True and each bank's copy-out +
store DMA pipeline against the next bank's tail matmul on alternating
rings.

fp8 was evaluated and rejected: e4m3 quantization of x and W measures
3.4e-2 full-output rel err, over the 2e-2 gate.
"""

import os
import sys

for _p in ("/opt/trn_rl_repo",):
    if _p not in sys.path:
        sys.path.insert(0, _p)

import numpy as np

import concourse.bass as bass
import concourse.bacc as bacc
import concourse.mybir as mybir
from concourse.tile import TileContext, add_dep_helper
from concourse.bass_utils import run_bass_kernel_spmd


def _ensure_ntff_hook_module():
    """run_bass_kernel_spmd(trace=True) imports antenv.axon_hooks, which the
    agent image's antenv package lacks. Provide it (and register the real
    ctypes NTFF hook when available) so a tracing caller doesn't crash."""
    import types

    try:
        import antenv
    except ImportError:
        return
    if getattr(antenv, "axon_hooks", None) is not None:
        return
    mod = types.ModuleType("antenv.axon_hooks")
    state = {"hook": None}
    mod.set_axon_ntff_profile_hook = lambda h: state.__setitem__("hook", h)
    mod.get_axon_ntff_profile_hook = lambda: state["hook"]
    sys.modules["antenv.axon_hooks"] = mod
    antenv.axon_hooks = mod
    try:
        from trn_agent_boot.trn_boot import _ntff_profile_via_ctypes

        mod.set_axon_ntff_profile_hook(
            _ntff_profile_via_ctypes("/opt/axon/libaxon_pjrt.so")
        )
    except Exception:
        pass


_ensure_ntff_hook_module()

B, IN, OUT = 512, 4096, 4096
NUM_CLASS, RANK = 64, 8
NCORES = 8
OUT_L = OUT // NCORES  # 512
P = 128
KT = IN // P           # 32 k-tiles
BT = B // P            # 4 batch tiles

NSING = 4                  # leading single-k-tile chunks (PE-side cushion:
                           # small chunks land fast, so the stream builds an
                           # SBUF backlog that rides out DMA jitter without
                           # PE gaps, which would drop the clock p-state)
NDBL = (KT - NSING) // 2   # 14 double-k-tile chunks

# single chunk columns: [x: B][w: OUT_L][a: 64]
S_XOFF, S_WOFF, S_AOFF = 0, B, B + OUT_L
SW = B + OUT_L + NUM_CLASS                 # 1088
# double chunk columns: [x0][x1][w0][w1][a0][a1]
D_XOFF, D_WOFF, D_AOFF = 0, 2 * B, 2 * (B + OUT_L)
DW = 2 * SW                                # 2176

# pp layout: [ps: BT*64][psT: B][bS: OUT_L (rows 0:65)]
PSOFF = 0
PTOFF = BT * NUM_CLASS
BSOFF = PTOFF + B
PPW = BSOFF + OUT_L

F32 = mybir.dt.float32
F32R = mybir.dt.float32r
F16 = mybir.dt.float16
X = mybir.AxisListType.X

NWARM = int(os.environ.get("NWARM", "6"))
PP_SLOT = int(os.environ.get("PP_SLOT", "6"))  # pp issued after this double

_cache = {}
# test.py reads this after a traced run for HW exec time
last_results = None


def _build():
    key = f"nc_w{NWARM}_p{PP_SLOT}"
    if key in _cache:
        return _cache[key]
    nc = bacc.Bacc(
        bass.get_trn_type() or "TRN2",
        target_bir_lowering=False,
        debug=False,
        num_devices=NCORES,
    )

    xw_s = nc.dram_tensor("xw_s", [NSING, P, SW], F16, kind="ExternalInput")
    xw_d = nc.dram_tensor("xw_d", [NDBL, P, DW], F16, kind="ExternalInput")
    pp = nc.dram_tensor("pp", [P, PPW], F32R, kind="ExternalInput")
    # f16 output staging: halves the PSUM->SBUF copy and store-DMA bytes on
    # the critical tail; the host upcasts. Adds ~2^-12 RMS rounding on top
    # of the fp16 stream's ~3e-4 rel err (measured total 3.9e-4).
    out = nc.dram_tensor("out", [B, OUT_L], F16, kind="ExternalOutput")

    with TileContext(nc) as tc:
        with (
            tc.tile_pool(name="xwp", bufs=1) as xwpool,
            tc.tile_pool(name="sml", bufs=1) as spool,
            tc.tile_pool(name="tl", bufs=1) as tpool,
            tc.tile_pool(name="op", bufs=1) as opool,
            tc.tile_pool(name="dr", bufs=1, space="DRAM") as dpool,
            tc.tile_pool(name="ps", bufs=1, space="PSUM") as ppool,
        ):
            # ---- PE warmup: dummy matmuls ramp the PE clock out of its
            # cold p-state while the first chunk DMA is in flight. The
            # result bank is never read.
            if NWARM:
                wt = spool.tile([P, P + OUT_L], F16, tag="warm")
                nc.vector.memset(wt, 0.0)
                warm_ps = ppool.tile([P, OUT_L], F32, tag="warm", name="warm")
                for _ in range(NWARM):
                    nc.tensor.matmul(
                        warm_ps,
                        lhsT=wt[:, :P],
                        rhs=wt[:, P : P + OUT_L],
                        start=True,
                        stop=True,
                    )

            # ---- input DMA triggers on the two HWDGE rings, window-2 gated:
            # each ring holds at most 2 in-flight transfers. With more, the
            # DMA queues round-robin across every outstanding transfer and
            # the FIRST chunk's completion slips by many us (measured: first
            # matmul at 18.6us instead of ~10us when all 17 were issued
            # up-front). pp is sequenced mid-stream on the scalar ring: its
            # consumers (top-2 mask, bS) aren't needed until stream end.
            s_tiles = [
                xwpool.tile([P, SW], F16, tag=f"s{c}", name=f"s{c}")
                for c in range(NSING)
            ]
            d_tiles = [
                xwpool.tile([P, DW], F16, tag=f"d{c}", name=f"d{c}")
                for c in range(NDBL)
            ]
            pp_sb = spool.tile([P, PPW], F32R)
            prev_dma = {0: None, 1: None}  # per-ring (c-2) gating chain
            gate_dma = {0: None, 1: None}

            def issue(eng_i, out_tile, src):
                eng = nc.sync if eng_i == 0 else nc.scalar
                dma = eng.dma_start(out=out_tile, in_=src)
                if gate_dma[eng_i] is not None:
                    add_dep_helper(
                        dma.ins,
                        gate_dma[eng_i].ins,
                        reason="window-2 DMA gating per ring",
                    )
                gate_dma[eng_i] = prev_dma[eng_i]
                prev_dma[eng_i] = dma
                return dma

            for s in range(NSING):
                issue(s % 2, s_tiles[s], xw_s[s])
            d_dmas = []
            for c in range(NDBL):
                d_dmas.append(issue(c % 2, d_tiles[c], xw_d[c]))
            # pp rides the GPSIMD SWDGE ring so it never crowds the chunk
            # stream; gated until d1 lands (its consumers run mid-stream)
            pp_dma = nc.gpsimd.dma_start(out=pp_sb, in_=pp[:, :])
            add_dep_helper(
                pp_dma.ins, d_dmas[1].ins, reason="keep pp off the early chunks"
            )

            ps_sb = pp_sb[:, PSOFF : PSOFF + BT * NUM_CLASS].bitcast(F32)
            psT_sb = pp_sb[:NUM_CLASS, PTOFF : PTOFF + B].bitcast(F32)
            bS_sb = pp_sb[:, BSOFF : BSOFF + OUT_L]  # rows 65:128 are zeros

            # ---- PSUM accumulators ----
            mps = [
                ppool.tile([P, OUT_L], F32, tag=f"main{bt}", name=f"main{bt}")
                for bt in range(BT)
            ]
            # G accumulator: full 128-partition bank; the real G lives in
            # rows 64:128. The G matmul's stationary is a 128-col window
            # ending at the a-block (64 w-cols of garbage + the 64 a-cols),
            # so its geometry matches the main matmuls exactly — a 64-wide
            # stationary forces a PE pipeline drain (~110ns) on the G matmul
            # AND on the following main (measured 333/322 vs 216ns).
            gt_ps = ppool.tile([P, B], F32, tag="gt", name="gt_ps")

            def do_k(xk, wk, a128, kidx):
                # G first: at the last k-tile this lets the DVE ht chain
                # overlap the final main matmuls
                nc.tensor.matmul(
                    gt_ps,
                    lhsT=a128,
                    rhs=xk,
                    start=(kidx == 0),
                    stop=(kidx == KT - 1),
                )
                for bt in range(BT):
                    nc.tensor.matmul(
                        mps[bt],
                        lhsT=xk[:, bt * P : (bt + 1) * P],
                        rhs=wk,
                        start=(kidx == 0),
                        stop=False,
                    )

            kidx = 0
            for s in range(NSING):
                t = s_tiles[s]
                do_k(
                    t[:, S_XOFF : S_XOFF + B],
                    t[:, S_WOFF : S_WOFF + OUT_L],
                    t[:, S_AOFF - NUM_CLASS : S_AOFF + NUM_CLASS],
                    kidx,
                )
                kidx += 1

            # ---- top-2 threshold + mask (DVE/GPSIMD, overlaps the stream) ----
            m2col = spool.tile([P, BT], F32)
            for bt in range(BT):
                pt = ps_sb[:, bt * NUM_CLASS : (bt + 1) * NUM_CLASS]
                m1 = spool.tile([P, 1], F32, tag=f"m1_{bt}")
                nc.vector.reduce_max(out=m1, in_=pt, axis=X)
                negmask = spool.tile([P, NUM_CLASS], F32, tag=f"nm_{bt}")
                # (pt >= m1) * -1e30  -> additive mask that kills the max
                nc.vector.tensor_scalar(
                    out=negmask,
                    in0=pt,
                    scalar1=m1,
                    scalar2=-1.0e30,
                    op0=mybir.AluOpType.is_ge,
                    op1=mybir.AluOpType.mult,
                )
                p2 = spool.tile([P, NUM_CLASS], F32, tag=f"p2_{bt}")
                nc.vector.tensor_tensor(
                    out=p2, in0=pt, in1=negmask, op=mybir.AluOpType.add
                )
                nc.vector.reduce_max(out=m2col[:, bt : bt + 1], in_=p2, axis=X)

            # threshold shuffle on the GPSIMD (SWDGE) path: partition->free
            # [128, BT] -> flat [B] via a DRAM bounce, then broadcast-read
            # across 64 partitions.
            m2d = dpool.tile([BT, P], F32)
            nc.gpsimd.dma_start(out=m2d.rearrange("bt p -> p bt"), in_=m2col[:, :])
            thr_sb = spool.tile([NUM_CLASS, B], F32)
            nc.gpsimd.dma_start(
                out=thr_sb,
                in_=m2d.rearrange("bt p -> (bt p)")[None, :].broadcast_to(
                    [NUM_CLASS, B]
                ),
            )
            thr2 = tpool.tile([NUM_CLASS, B], F32)
            nc.vector.tensor_copy(out=thr2, in_=thr_sb)
            psT2 = tpool.tile([NUM_CLASS, B], F32)
            nc.vector.tensor_copy(out=psT2, in_=psT_sb)
            maskT = tpool.tile([NUM_CLASS, B], F32)
            nc.vector.tensor_tensor(
                out=maskT, in0=psT2, in1=thr2, op=mybir.AluOpType.is_ge
            )
            # ht padded to a full 128-row contraction (rows 65:128 zero, with
            # matching zero rows in bS) so the tail matmul keeps the same
            # 128-partition geometry as the stream and avoids the PE drain.
            ht = tpool.tile([P, B], F32R)
            # zero rows 64:128 (aligned base), then the ones row at 64
            # (Memset can't write float32r, hence the in0*0+c idiom)
            nc.vector.tensor_scalar(
                out=ht[NUM_CLASS:P, :],
                in0=psT2[0 : P - NUM_CLASS, :],
                scalar1=0.0,
                scalar2=0.0,
                op0=mybir.AluOpType.mult,
                op1=mybir.AluOpType.add,
            )
            nc.vector.tensor_scalar(
                out=ht[NUM_CLASS : NUM_CLASS + 1, :],
                in0=thr2[0:1, :],
                scalar1=0.0,
                scalar2=1.0,
                op0=mybir.AluOpType.mult,
                op1=mybir.AluOpType.add,
            )

            # ---- double chunks ----
            for c in range(NDBL):
                t = d_tiles[c]
                for kk in range(2):
                    do_k(
                        t[:, D_XOFF + kk * B : D_XOFF + (kk + 1) * B],
                        t[:, D_WOFF + kk * OUT_L : D_WOFF + (kk + 1) * OUT_L],
                        t[
                            :,
                            D_AOFF
                            + (kk - 1) * NUM_CLASS : D_AOFF
                            + (kk + 1) * NUM_CLASS,
                        ],
                        kidx,
                    )
                    kidx += 1

            # ht = G * mask, reading the G accumulator PSUM directly (the
            # real G sits in rows 64:128 of the padded accumulator)
            nc.vector.tensor_tensor(
                out=ht[0:NUM_CLASS, :],
                in0=gt_ps[NUM_CLASS : 2 * NUM_CLASS, :],
                in1=maskT,
                op=mybir.AluOpType.mult,
            )

            # ---- tail + epilogue, pipelined per batch tile: the rank-65
            # LoRA matmul closes bank bt, whose copy-out + store DMA then
            # overlap the next bank's tail matmul.
            o_all = opool.tile([P, BT * OUT_L], F16)
            for bt in range(BT):
                nc.tensor.matmul(
                    mps[bt],
                    lhsT=ht[:, bt * P : (bt + 1) * P],
                    rhs=bS_sb,
                    start=False,
                    stop=True,
                )
                nc.vector.tensor_copy(
                    out=o_all[:, bt * OUT_L : (bt + 1) * OUT_L], in_=mps[bt]
                )
                # spread store-DMA triggers across three engines so their
                # ~0.6us issue costs run in parallel
                eng = (nc.sync, nc.scalar, nc.gpsimd, nc.sync)[bt]
                eng.dma_start(
                    out=out[bt * P : (bt + 1) * P, :],
                    in_=o_all[:, bt * OUT_L : (bt + 1) * OUT_L],
                )

    nc.finalize()
    _cache[key] = nc
    return nc


def _pack_inputs(x, pseudo_index, weight, bias, lora_A, lora_B):
    """Build the per-core chunked xw buffers + replicated small inputs."""
    xT = np.ascontiguousarray(x.T).astype(np.float16)        # [IN, B]
    aT = np.ascontiguousarray(
        lora_A[:NUM_CLASS].T
    ).astype(np.float16)                                     # [IN, 64]

    pp_base = np.zeros((P, PPW), dtype=np.float32)
    pp_base[:, PSOFF : PSOFF + BT * NUM_CLASS] = (
        pseudo_index.reshape(BT, P, NUM_CLASS)
        .transpose(1, 0, 2)
        .reshape(P, BT * NUM_CLASS)
    )
    pp_base[:NUM_CLASS, PTOFF : PTOFF + B] = pseudo_index.T

    in_maps = []
    for i in range(NCORES):
        o0 = i * OUT_L
        wTi = weight[o0 : o0 + OUT_L].T.astype(np.float16)   # [IN, OUT_L]

        xws = np.empty((NSING, P, SW), dtype=np.float16)
        for s in range(NSING):
            xws[s, :, S_XOFF : S_XOFF + B] = xT[s * P : (s + 1) * P]
            xws[s, :, S_WOFF : S_WOFF + OUT_L] = wTi[s * P : (s + 1) * P]
            xws[s, :, S_AOFF : S_AOFF + NUM_CLASS] = aT[s * P : (s + 1) * P]

        xwd = np.empty((NDBL, P, DW), dtype=np.float16)
        for c in range(NDBL):
            k0 = NSING + 2 * c
            k1 = k0 + 1
            xwd[c, :, D_XOFF : D_XOFF + B] = xT[k0 * P : (k0 + 1) * P]
            xwd[c, :, D_XOFF + B : D_XOFF + 2 * B] = xT[k1 * P : (k1 + 1) * P]
            xwd[c, :, D_WOFF : D_WOFF + OUT_L] = wTi[k0 * P : (k0 + 1) * P]
            xwd[c, :, D_WOFF + OUT_L : D_WOFF + 2 * OUT_L] = wTi[
                k1 * P : (k1 + 1) * P
            ]
            xwd[c, :, D_AOFF : D_AOFF + NUM_CLASS] = aT[k0 * P : (k0 + 1) * P]
            xwd[c, :, D_AOFF + NUM_CLASS : DW] = aT[k1 * P : (k1 + 1) * P]

        ppi = pp_base.copy()
        ppi[:NUM_CLASS, BSOFF : BSOFF + OUT_L] = (
            16.0 * lora_B[o0 : o0 + OUT_L, :NUM_CLASS].T
        )
        ppi[NUM_CLASS, BSOFF : BSOFF + OUT_L] = 2.0 * bias[o0 : o0 + OUT_L]
        in_maps.append({"xw_s": xws, "xw_d": xwd, "pp": ppi})
    return in_maps


def kernel(x, pseudo_index, weight, bias, lora_A, lora_B):
    global last_results
    x = np.ascontiguousarray(np.asarray(x, dtype=np.float32))
    pseudo_index = np.ascontiguousarray(np.asarray(pseudo_index, dtype=np.float32))
    weight = np.asarray(weight, dtype=np.float32)
    bias = np.asarray(bias, dtype=np.float32)
    lora_A = np.asarray(lora_A, dtype=np.float32)
    lora_B = np.asarray(lora_B, dtype=np.float32)

    nc = _build()
    in_maps = _pack_inputs(x, pseudo_index, weight, bias, lora_A, lora_B)
    res = run_bass_kernel_spmd(nc, in_maps, list(range(NCORES)))
    last_results = res
    return np.hstack(
        [res.results[i]["out"].astype(np.float32) for i in range(NCORES)]
    )
